# revision 33
# baseline (speedup 1.0000x reference)
"""Causal attention layer (B=4, N=2048, C=1024, H=16, D=64) on 8 TRN2 NeuronCores.

Sharding: core c -> (batch b = c//2, head-group g = c%2 of 8 heads).
Per core, for its (b, g):
  qkvT  = wqkvT_g.T-contract(x_b)      q,k stored [ch, n] bf16; v transposed via
                                       PE into v_ext [kn, 1|v] (ones col first)
  S_T   = kT.T @ qT                    pre-transposed scores [kn, qn]; masked q
                                       columns of diagonal tiles are skipped
  P_T   = exp(S_T/8) bf16              diagonal 128-tiles masked by tril multiply
  AV    = P_T-slice.T @ [1|v]          stationary = P^T per q-tile (full 128-row
                                       output); col 0 accumulates the denominator
  attn  = AV[:,1:65] * (1/AV[:,0])     per-partition scalar multiply, then PE
                                       transpose back to [ch, n] for the proj
  out   = attn_T.T-contract(projT_g)   bf16 partial written to HBM
Host sums the two head-group partials per batch and adds proj_b.

The emission stream is software-pipelined: x/w DMAs are issued per-super so the
first matmuls start ~4us in; all later qkv o-tiles, v transposes and the output
projection are queued as fillers interleaved into the attention stream so the
PE stays dense while ScalarE runs exp.
"""
import sys

sys.path.insert(0, "/opt/trn_rl_repo")

import numpy as np

import concourse.bass as bass  # noqa: F401
import concourse.tile as tile
from concourse import bacc, mybir
from concourse.bass_utils import run_bass_kernel_spmd

F32 = mybir.dt.float32
BF16 = mybir.dt.bfloat16
EXP = mybir.ActivationFunctionType.Exp

B, N, C, H, D = 4, 2048, 1024, 16, 64
G = 8            # heads per core
GC = G * D       # 512 channels per core
NT = N // 128    # 16 row tiles
NS = N // 512    # 4 row supers
CK = C // 128    # 8 contraction chunks

_cache = {}


def _build_nc():
    from contextlib import ExitStack

    nc = bacc.Bacc("TRN2", target_bir_lowering=False, debug=False)

    xT_d = nc.dram_tensor("xT", [CK, 128, N], BF16, kind="ExternalInput")
    w_d = nc.dram_tensor("wq", [12, 128, CK, 128], BF16, kind="ExternalInput")
    pj_d = nc.dram_tensor("pj", [4, 128, C], BF16, kind="ExternalInput")
    tril_d = nc.dram_tensor("tril", [128, 128], BF16, kind="ExternalInput")
    ident_d = nc.dram_tensor("ident", [128, 128], BF16, kind="ExternalInput")
    out_d = nc.dram_tensor("out", [N, C], BF16, kind="ExternalOutput")

    with tile.TileContext(nc) as tc:
        with ExitStack() as ctx:
            consts = ctx.enter_context(tc.tile_pool(name="consts", bufs=1))
            qk_pool = ctx.enter_context(tc.tile_pool(name="qk", bufs=4))
            vext_pool = ctx.enter_context(tc.tile_pool(name="vext", bufs=1))
            vT_pool = ctx.enter_context(tc.tile_pool(name="vT", bufs=2))
            w_pool = ctx.enter_context(tc.tile_pool(name="wA", bufs=1))
            xT_pool = ctx.enter_context(tc.tile_pool(name="xT", bufs=1))
            rf_pool = ctx.enter_context(tc.tile_pool(name="rf", bufs=2))
            bcs_pool = ctx.enter_context(tc.tile_pool(name="bcs", bufs=2))
            tmp_pool = ctx.enter_context(tc.tile_pool(name="tmp", bufs=2))
            ob_pool = ctx.enter_context(tc.tile_pool(name="ob", bufs=2))
            pj_pool = ctx.enter_context(tc.tile_pool(name="pj", bufs=1))
            psA = ctx.enter_context(tc.tile_pool(name="psA", bufs=3, space="PSUM"))

            tril_sb = consts.tile([128, 128], BF16)
            nc.sync.dma_start(tril_sb[:], tril_d[:])
            ident_sb = consts.tile([128, 128], BF16)
            nc.sync.dma_start(ident_sb[:], ident_d[:])

            all_w = {}

            def _wdma(ots):
                for ot in ots:
                    wt = w_pool.tile([128, CK, 128], BF16, tag=f"wt{ot}",
                                     name=f"wt{ot}")
                    nc.sync.dma_start(wt[:], w_d[ot])
                    all_w[ot] = wt

            def load_w(ot):
                return all_w[ot]

            # v_ext[h]: [128 keys, NT, 128]; col 0 = ones (softmax denom),
            # cols 64:128 = v^T (32-aligned partition offsets downstream)
            v_ext = [vext_pool.tile([128, NT, 128], BF16, tag=f"ve{h}", name=f"ve{h}")
                     for h in range(G)]
            for h in range(G):
                nc.vector.memset(v_ext[h][:, :, 0:1], 1.0)
                nc.vector.memset(v_ext[h][:, :, 1:64], 0.0)

            # x streamed per (super, chunk) so compute can start early;
            # weight dmas interleaved between supers so sup0 lands first
            xs = [xT_pool.tile([128, N], BF16, tag=f"x{cc}", name=f"x{cc}")
                  for cc in range(CK)]
            # w8/w0/w4 land inside sup0's stream so the first three
            # quarters never wait on a weight dma
            wsched = {1: (9, 1, 5), 2: (10, 2, 6), 3: (11, 3, 7)}
            w0sched = {0: (8,), 3: (0,), 5: (4,)}
            for sup in range(NS):
                for cc in range(CK):
                    nc.sync.dma_start(
                        xs[cc][:, 512 * sup:512 * (sup + 1)],
                        xT_d[cc, :, 512 * sup:512 * (sup + 1)],
                    )
                    if sup == 0 and cc in w0sched:
                        _wdma(w0sched[cc])
                if sup + 1 in wsched:
                    _wdma(wsched[sup + 1])

            pj_sb = [pj_pool.tile([128, C], BF16, tag=f"pj{i}", name=f"pj{i}")
                     for i in range(4)]
            for ac in range(4):
                nc.sync.dma_start(pj_sb[ac][:], pj_d[ac])

            def qkv_quarter(wt, sup):
                psq = psA.tile([128, 512], F32, tag="qa", name="psq")
                for cc in range(CK):
                    nc.tensor.matmul(
                        psq[:],
                        wt[:, cc, :],
                        xs[cc][:, 512 * sup:512 * (sup + 1)],
                        start=(cc == 0),
                        stop=(cc == CK - 1),
                    )
                return psq

            # ------- step builders (emitted inline or queued as fillers) --------
            # Each step is {"u": units, "fn": emit, "done": bool}. Weight loads
            # are bound per-group via a closure box so interleaved groups don't
            # clobber each other's stationary tile.
            def _step(units, fn):
                return {"u": units, "fn": fn, "done": False}

            def v_steps_full(vp):
                vt = vT_pool.tile([128, N], BF16, tag="vt", name=f"vt{vp}")
                box = {}

                def _w(vp=vp):
                    box["wt"] = load_w(8 + vp)
                steps = [_step(0, _w)]
                for sup in range(NS):
                    def _mms(vt=vt, sup=sup):
                        psq = qkv_quarter(box["wt"], sup)
                        nc.vector.tensor_copy(
                            vt[:, 512 * sup:512 * (sup + 1)], psq[:]
                        )
                    steps.append(_step(4, _mms))
                for nt in range(NT):
                    def _tr(vt=vt, vp=vp, nt=nt):
                        tp = psA.tile([128, 128], BF16, tag="qa", name="tp")
                        nc.tensor.transpose(
                            tp[:], vt[:, 128 * nt:128 * (nt + 1)], ident_sb[:]
                        )
                        nc.vector.tensor_copy(
                            v_ext[2 * vp][:, nt, 64:128], tp[:, 0:64]
                        )
                        nc.vector.tensor_copy(
                            v_ext[2 * vp + 1][:, nt, 64:128], tp[:, 64:128]
                        )
                    steps.append(_step(0, _tr))
                return steps

            def qk_steps(p):
                qT = qk_pool.tile([128, N], BF16, tag="qk", name=f"q{p}")
                kT = qk_pool.tile([128, N], BF16, tag="qk", name=f"k{p}")
                qsteps, ksteps = [], []
                for dst, ot, steps in ((qT, p, qsteps), (kT, 4 + p, ksteps)):
                    box = {}

                    def _w(box=box, ot=ot):
                        box["wt"] = load_w(ot)
                    steps.append(_step(0, _w))
                    for sup in range(NS):
                        def _mms(box=box, dst=dst, sup=sup):
                            psq = qkv_quarter(box["wt"], sup)
                            nc.vector.tensor_copy(
                                dst[:, 512 * sup:512 * (sup + 1)], psq[:]
                            )
                        steps.append(_step(4, _mms))
                return qT, kT, qsteps, ksteps

            attn_outT = None

            def proj_steps(s):
                steps = []
                for nt in range(4 * s, 4 * s + 4):
                    for oc in (0, 1):
                        def _pj(nt=nt, oc=oc):
                            pp2 = psA.tile([128, 512], F32, tag="qa", name="pp2")
                            for ac in range(4):
                                nc.tensor.matmul(
                                    pp2[:],
                                    attn_outT[ac][:, 128 * nt:128 * (nt + 1)],
                                    pj_sb[ac][:, 512 * oc:512 * (oc + 1)],
                                    start=(ac == 0),
                                    stop=(ac == 3),
                                )
                            ob = ob_pool.tile([128, 512], BF16, tag="ob", name="ob")
                            nc.vector.tensor_copy(ob[:], pp2[:])
                            nc.sync.dma_start(
                                out_d[128 * nt:128 * (nt + 1),
                                      512 * oc:512 * (oc + 1)],
                                ob[:],
                            )
                        steps.append(_step(2, _pj))
                return steps

            # filler machinery with adaptive pacing: each attention half-step
            # releases credit = remaining filler units / remaining halves, so
            # the filler queue is spread evenly over the whole attention span.
            pending = []
            pace = {"units": 0, "halves": 320, "credit": 0.0}

            def run_step(st):
                st["fn"]()
                st["done"] = True

            def queue(steps):
                pending.extend(steps)
                pace["units"] += sum(st["u"] for st in steps)

            def _pop():
                st = pending.pop(0)
                pace["units"] -= st["u"]
                run_step(st)
                return st

            def fill_half():
                # called once per (kg, h) attention half
                if pace["halves"] > 1:
                    pace["credit"] += 0.9 * pace["units"] / pace["halves"]
                pace["halves"] = max(1, pace["halves"] - 1)
                while pending and pace["credit"] >= pending[0]["u"]:
                    st = _pop()
                    pace["credit"] -= st["u"]

            def fill_all():
                while pending:
                    _pop()

            def drain(st):
                # force-emit everything up to and including step st (FIFO)
                while not st["done"]:
                    _pop()

            # ---------------- prologue: v0/q0/k0 sup0 only ----------------------
            vgrp = {0: v_steps_full(0)}
            # run weight + sup0 quarter + first 4 transposes inline
            for st in (vgrp[0][0], vgrp[0][1], vgrp[0][5], vgrp[0][6],
                       vgrp[0][7], vgrp[0][8]):
                run_step(st)

            qT, kT, q0s, k0s = qk_steps(0)
            qgrp, kgrp = {0: q0s}, {0: k0s}
            # run wq, q_sup0, wk, k_sup0 inline
            for st in (q0s[0], q0s[1], k0s[0], k0s[1]):
                run_step(st)
            # interleave remaining qk0/v0 supers so each super's q/k/v_ext is
            # ready when the attention loop reaches it
            for sup in (1, 2, 3):
                queue([vgrp[0][1 + sup]])
                queue(vgrp[0][9 + 4 * (sup - 1):9 + 4 * sup])
                queue([k0s[1 + sup], q0s[1 + sup]])

            # ---------------- attention pair loop (with fillers) ----------------
            with (
                tc.tile_pool(name="aoT", bufs=1) as aoT_pool,
                tc.tile_pool(name="pt", bufs=7) as pt_pool,
                tc.tile_pool(name="psS", bufs=3, space="PSUM") as psS,
                tc.tile_pool(name="psV", bufs=2, space="PSUM") as psV,
            ):
                attn_outT = [aoT_pool.tile([128, N], BF16, tag=f"ao{p}", name=f"ao{p}")
                             for p in range(4)]
                for p in range(4):
                    if p < 3:
                        vgrp[p + 1] = v_steps_full(p + 1)
                        nq, nk_, nqs, nks = qk_steps(p + 1)
                        qgrp[p + 1], kgrp[p + 1] = nqs, nks
                        queue(vgrp[p + 1])
                        queue(nqs)
                        queue(nks)

                    sorder = (3, 0, 1, 2) if p == 3 else range(NS)
                    for s in sorder:
                        # correctness: q/k quarters for this super and v_ext
                        # transposes for all key tiles of this super must be
                        # emitted before their readers below
                        drain(qgrp[p][1 + s])
                        drain(kgrp[p][1 + s])
                        drain(vgrp[p][5 + min(4 * s + 3, NT - 1)])
                        if 0 < s < 3:
                            # pull next super's q/k casts a window early so
                            # scores don't wait on just-emitted DVE casts
                            drain(qgrp[p][2 + s])
                            drain(kgrp[p][2 + s])
                        nkb = 4 * (s + 1)
                        oT = [psV.tile([128, 512], F32, tag="ot",
                                       name=f"ot{p}{s}{h}") for h in (0, 1)]
                        avq = []  # AV emission deferred 4 halves behind exp
                        for ki in range(nkb):
                            ridx = ki - 4 * s
                            off = 128 * ridx if ridx > 0 else 0
                            for h in (0, 1):
                                fill_half()
                                hh = slice(64 * h, 64 * (h + 1))
                                S2 = psS.tile([128, 512], F32, tag="s2", name="S2")
                                nc.tensor.matmul(
                                    S2[:, off:512],
                                    kT[hh, 128 * ki:128 * (ki + 1)],
                                    qT[hh, 512 * s + off:512 * (s + 1)],
                                )
                                P2 = pt_pool.tile([128, 512], BF16, tag="pt",
                                                  name="P2")
                                nc.scalar.activation(
                                    P2[:, off:512], S2[:, off:512],
                                    EXP, scale=float(D) ** -0.5,
                                )
                                if ridx >= 0:
                                    # diagonal 128-tile: causal mask via tril
                                    nc.vector.tensor_mul(
                                        P2[:, off:off + 128],
                                        P2[:, off:off + 128],
                                        tril_sb[:],
                                    )

                                # AV: stationary [1|v], moving P^T; masked q
                                # columns of diagonal tiles are never read, so
                                # no memsets are needed
                                def _av(P2=P2, ki=ki, h=h, off=off, nkb=nkb,
                                        oT=oT, p=p):
                                    nc.tensor.matmul(
                                        oT[h][:, off:512],
                                        v_ext[2 * p + h][:, ki, :],
                                        P2[:, off:512],
                                        start=(ki == 0),
                                        stop=(ki == nkb - 1),
                                    )
                                avq.append(_av)
                                if len(avq) > 4:
                                    avq.pop(0)()
                        for av in avq:
                            av()
                        # ---- normalize (row 0 = denominators) + write back --
                        for h in (0, 1):
                            rf = rf_pool.tile([1, 512], F32, tag="rf", name="rf")
                            nc.vector.reciprocal_approx_fast(rf[:], oT[h][0:1, :])
                            bcs = bcs_pool.tile([128, 512], F32, tag="bc",
                                                name="bc")
                            nc.gpsimd.partition_broadcast(bcs[:], rf[:])
                            tmp = tmp_pool.tile([128, 512], BF16, tag="tm",
                                                name="tm")
                            nc.vector.tensor_mul(tmp[:], oT[h][:], bcs[:])
                            nc.sync.dma_start(
                                attn_outT[p][64 * h:64 * (h + 1),
                                             512 * s:512 * (s + 1)],
                                tmp[64:128, :],
                            )
                        if p == 3:
                            queue(proj_steps(s))
                    if p < 3:
                        qT, kT = nq, nk_
                fill_all()

    nc.compile()
    return nc


def _tril_np():
    import ml_dtypes

    i = np.arange(128)[:, None]
    j = np.arange(128)[None, :]
    return (j >= i).astype(np.float32).astype(ml_dtypes.bfloat16)


def make_in_maps(x, qkv_w, proj_w):
    import ml_dtypes

    bf16 = ml_dtypes.bfloat16
    x = np.asarray(x, dtype=np.float32)
    qkv_w = np.asarray(qkv_w, dtype=np.float32)
    proj_w = np.asarray(proj_w, dtype=np.float32)
    tril = _tril_np()
    ident = np.eye(128, dtype=np.float32).astype(bf16)
    in_maps = []
    for c in range(8):
        b, g = c // 2, c % 2
        sl = slice(g * GC, (g + 1) * GC)
        wq, wk, wv = qkv_w[0:C][sl], qkv_w[C:2 * C][sl], qkv_w[2 * C:3 * C][sl]
        wcat = np.concatenate([wq, wk, wv], 0).T  # [C, 1536]
        w12 = np.stack(
            [
                np.ascontiguousarray(
                    wcat[:, 128 * ot:128 * (ot + 1)]
                    .reshape(CK, 128, 128)
                    .transpose(1, 0, 2)
                )
                for ot in range(12)
            ],
            0,
        ).astype(bf16)
        xT = np.ascontiguousarray(x[b].T).astype(bf16).reshape(CK, 128, N)
        pj = (
            np.ascontiguousarray(proj_w[:, sl].T)
            .astype(bf16)
            .reshape(4, 128, C)
        )
        in_maps.append(
            {"xT": xT, "wq": w12, "pj": pj, "tril": tril, "ident": ident}
        )
    return in_maps


def kernel(x, qkv_w, proj_w, proj_b):
    proj_b = np.asarray(proj_b, dtype=np.float32)

    if "nc" not in _cache:
        _cache["nc"] = _build_nc()
    nc = _cache["nc"]

    in_maps = make_in_maps(x, qkv_w, proj_w)
    res = run_bass_kernel_spmd(nc, in_maps, core_ids=list(range(8)))
    out = np.stack(
        [
            res.results[2 * b]["out"].astype(np.float32)
            + res.results[2 * b + 1]["out"].astype(np.float32)
            for b in range(B)
        ],
        0,
    )
    return (out + proj_b[None, None, :]).astype(np.float32)
